# revision 7
# baseline (speedup 1.0000x reference)
"""Cross-attention kernel for Trainium2 (8 NeuronCores).

Sharding: 8 cores = 4 batches x 2 head-groups (8 of 16 heads each).
Per core (batch b, group g):
    Q^T = (Wq_g)^T x_q^T          [512, 2048]  (bf16 matmuls, fp32 psum)
    K^T = (Wk_g)^T x_k^T          [512, 2048]
    V   = x_v Wv_g                [2048, 512]
    S^T = K^T(h)^T-contracted scores per head, exp on ACT with
          scale=1/8 and per-k bias c_k = (K bq + bq.bk)/8
    PV^T accumulated per head pair via PE col-tiling, softmax sums via
    ones-matmuls, normalization = DVE recip + gpsimd partition_broadcast.
    yT  = (headoutN @ Wo_g)^T     [1024, 2048]  (partial, pre-bias)
Host: out[b] = (yT[2b] + yT[2b+1]).T + bo + bv @ Wo.
bk's per-q score shift cancels in softmax exactly; bq enters via c_k.

Input activations are transposed on-device: f32 -> bf16 (DVE/ACT), staged
to DRAM, then DMA-transposed back into SBUF.
"""

from contextlib import ExitStack

import numpy as np

import concourse.bass as bass
import concourse.tile as tile
from concourse import bacc, mybir
from concourse import bass_utils
from concourse.bass import ts, ds

F32 = mybir.dt.float32
BF16 = mybir.dt.bfloat16
AF = mybir.ActivationFunctionType

S = 2048          # query and key sequence length
E = 1024          # embed dim
C = 512           # per-core projection cols (8 heads x 64)
HL = 8            # local heads
D = 64            # head dim
NKT = S // 128    # 16 key tiles
NST = S // 128    # 16 seq tiles
NET = E // 128    # 8 embed tiles
NCT = C // 128    # 4 proj-col tiles (= head pairs)
QCH = 1024        # q chunk width in attention
NQC = S // QCH    # 2
N_CORES = 8


def build_program(nc):
    xq = nc.dram_tensor("xq", [S, E], F32, kind="ExternalInput").ap()
    xk = nc.dram_tensor("xk", [S, E], F32, kind="ExternalInput").ap()
    xv = nc.dram_tensor("xv", [S, E], F32, kind="ExternalInput").ap()
    wq = nc.dram_tensor("wq", [E, C], F32, kind="ExternalInput").ap()
    wk = nc.dram_tensor("wk", [E, C], F32, kind="ExternalInput").ap()
    wv = nc.dram_tensor("wv", [E, C], F32, kind="ExternalInput").ap()
    wo = nc.dram_tensor("wo", [C, E], F32, kind="ExternalInput").ap()
    bq4 = nc.dram_tensor("bq4", [128, NCT], F32, kind="ExternalInput").ap()
    bqbk8 = nc.dram_tensor("bqbk8", [128, HL], F32, kind="ExternalInput").ap()
    yT = nc.dram_tensor("yT", [E, S], F32, kind="ExternalOutput").ap()

    with tile.TileContext(nc) as tc, ExitStack() as ctx:
            pool = lambda **kw: ctx.enter_context(tc.tile_pool(**kw))
            xstage = pool(name="xstage", bufs=2)
            xbfp = pool(name="xbf", bufs=2)
            dramp = pool(name="dstage", bufs=3, space="DRAM")
            xTp = pool(name="xT", bufs=8)
            wstage = pool(name="wstage", bufs=2)
            wbfp = pool(name="wbf", bufs=8)
            wobfp = pool(name="wobf", bufs=4)
            kqvp = pool(name="kqv", bufs=8)
            vpool = pool(name="vpool", bufs=16)
            honp = pool(name="hon", bufs=4)
            ckp = pool(name="ckp", bufs=8)
            ptp = pool(name="pt", bufs=2)
            npool = pool(name="norm", bufs=1)
            rpool = pool(name="rep", bufs=1)
            outp = pool(name="outp", bufs=2)
            constp = pool(name="const", bufs=1)
            psp = pool(name="ps", bufs=2, space="PSUM")
            accp = pool(name="acc", bufs=2, space="PSUM")
            sumsp = pool(name="sums", bufs=1, space="PSUM")
            # constants; also pre-warm the exp table so the ~2.7us
            # ACT_TABLE_LOAD is off the critical path
            ones = constp.tile([128, 1], BF16, tag="ones")
            nc.gpsimd.memset(ones[:], 1.0)
            warm = constp.tile([128, 1], F32, tag="warm")
            nc.gpsimd.memset(warm[:], 0.0)
            wexp = constp.tile([128, 1], F32, tag="wexp")
            nc.scalar.activation(wexp[:], warm[:], AF.Exp)

            bq4f = constp.tile([128, NCT], F32, tag="bq4f")
            nc.sync.dma_start(bq4f[:], bq4[:])
            bq4b = constp.tile([128, NCT], BF16, tag="bq4b")
            nc.vector.tensor_copy(bq4b[:], bq4f[:])
            bqbk = constp.tile([128, HL], F32, tag="bqbk")
            nc.sync.dma_start(bqbk[:], bqbk8[:])

            def load_w(w_ap, tag):
                tiles = []
                for i in range(NET):
                    wf = wstage.tile([128, C], F32, tag="wstage")
                    nc.sync.dma_start(wf[:], w_ap[ts(i, 128), :])
                    wb = wbfp.tile([128, C], BF16, tag=tag)
                    nc.vector.tensor_copy(wb[:], wf[:])
                    tiles.append(wb)
                return tiles

            def stage_transpose(x_ap, tag):
                """x [S, E] f32 DRAM -> list of NET [128, S] bf16 x^T tiles."""
                xd = dramp.tile([S, E], BF16, tag=f"{tag}d")
                for i in range(NST):
                    xf = xstage.tile([128, E], F32, tag="xf32")
                    nc.sync.dma_start(xf[:], x_ap[ts(i, 128), :])
                    xb = xbfp.tile([128, E], BF16, tag="xbf")
                    if i % 2 == 0:
                        nc.vector.tensor_copy(xb[:], xf[:])
                    else:
                        nc.scalar.copy(xb[:], xf[:])
                    nc.sync.dma_start(xd[ts(i, 128), :], xb[:])
                xt = []
                for j in range(NET):
                    t = xTp.tile([128, S], BF16, tag="xT")
                    nc.sync.dma_start_transpose(out=t[:], in_=xd[:, ts(j, 128)])
                    xt.append(t)
                return xt

            def proj_T(xt, w_bf, tag):
                """out[c, s] = sum_e W[e, c] x^T[e, s] -> NCT tiles [128, S] bf16."""
                outs = []
                for m in range(NCT):
                    t = kqvp.tile([128, S], BF16, tag=tag)
                    for nch in range(S // 512):
                        p = psp.tile([128, 512], F32, tag="ps")
                        for k in range(NET):
                            nc.tensor.matmul(
                                p[:],
                                lhsT=w_bf[k][:, ts(m, 128)],
                                rhs=xt[k][:, ts(nch, 512)],
                                start=(k == 0),
                                stop=(k == NET - 1),
                            )
                        nc.scalar.copy(t[:, ts(nch, 512)], p[:])
                    outs.append(t)
                return outs

            # ---- K pipeline ----
            wk_bf = load_w(wk, "wkb")
            xkt = stage_transpose(xk, "xk")
            KT = proj_T(xkt, wk_bf, "KT")

            # ---- c_k bias columns: 0.125*(K_nb^T bq) + 0.125*bq.bk ----
            ck_sb = []
            for h in range(HL):
                j, hp = divmod(h, 2)
                pb = hp * 64
                p = psp.tile([128, NKT], F32, tag="ps")
                for kt in range(NKT):
                    nc.tensor.matmul(
                        p[:, kt : kt + 1],
                        lhsT=KT[j][pb : pb + 64, ts(kt, 128)],
                        rhs=bq4b[pb : pb + 64, j : j + 1],
                        start=True,
                        stop=True,
                        tile_position=(pb, 0),
                    )
                t = ckp.tile([128, NKT], F32, tag="ck")
                nc.scalar.activation(
                    t[:], p[:], AF.Identity, bias=bqbk[:, h : h + 1], scale=0.125
                )
                ck_sb.append(t)

            # ---- Q pipeline ----
            wq_bf = load_w(wq, "wqb")
            xqt = stage_transpose(xq, "xq")
            QT = proj_T(xqt, wq_bf, "QT")

            # ---- V pipeline (natural orientation) ----
            wv_bf = load_w(wv, "wvb")
            xvt = stage_transpose(xv, "xv")
            v_tiles = []
            for st in range(NST):
                p = psp.tile([128, C], F32, tag="ps")
                for k in range(NET):
                    nc.tensor.matmul(
                        p[:],
                        lhsT=xvt[k][:, ts(st, 128)],
                        rhs=wv_bf[k][:],
                        start=(k == 0),
                        stop=(k == NET - 1),
                    )
                vt = vpool.tile([128, HL * 65], BF16, tag="v")
                nc.gpsimd.memset(vt[:], 1.0)
                for h in range(HL):
                    nc.vector.tensor_copy(
                        vt[:, ds(h * 65, 64)], p[:, ds(h * 64, 64)]
                    )
                v_tiles.append(vt)

            # ---- Wo ----
            wo_bf = []
            for i in range(NCT):
                wf = xstage.tile([128, E], F32, tag="xf32")
                nc.sync.dma_start(wf[:], wo[ts(i, 128), :])
                wb = wobfp.tile([128, E], BF16, tag="wob")
                nc.vector.tensor_copy(wb[:], wf[:])
                wo_bf.append(wb)

            hoN = [
                honp.tile([128, S], BF16, tag="hoN", name=f"hoN{i}")
                for i in range(NCT)
            ]

            # ---- attention ----
            for qc in range(NQC):
                for j in range(NCT):
                  for hp in range(2):
                    h = 2 * j + hp
                    pb = hp * 64
                    a = accp.tile([128, QCH], F32, tag="acc", name=f"a{qc}_{h}")
                    for kt in range(NKT):
                        first, last = kt == 0, kt == NKT - 1
                        s0 = psp.tile([128, QCH], F32, tag="ps", name=f"s{qc}_{h}_{kt}")
                        for half in range(2):
                            qs = ds(qc * QCH + half * 512, 512)
                            nc.tensor.matmul(
                                s0[:, ts(half, 512)],
                                lhsT=KT[j][pb : pb + 64, ts(kt, 128)],
                                rhs=QT[j][pb : pb + 64, qs],
                                start=True, stop=True,
                                tile_position=(pb, 0),
                            )
                        p0 = ptp.tile([128, QCH], BF16, tag="pt", name=f"p{qc}_{h}_{kt}")
                        nc.scalar.activation(
                            p0[:], s0[:], AF.Exp,
                            bias=ck_sb[h][:, kt : kt + 1], scale=0.125,
                        )
                        for half in range(2):
                            hs = ts(half, 512)
                            nc.tensor.matmul(
                                a[0:65, hs],
                                lhsT=v_tiles[kt][:, ds(h * 65, 65)],
                                rhs=p0[:, hs],
                                start=first, stop=last,
                                tile_position=(0, 0),
                            )
                    rc = npool.tile([128, QCH], F32, tag="rc", name=f"rc{qc}_{h}")
                    nc.vector.reciprocal(rc[64:65, :], a[64:65, :])
                    nc.vector.tensor_copy(rc[0:1, :], rc[64:65, :])
                    rp = rpool.tile([128, QCH], F32, tag="rp", name=f"rp{qc}_{h}")
                    nc.gpsimd.partition_broadcast(rp[0:64, :], rc[0:1, :])
                    if hp == 0:
                        nc.vector.tensor_mul(
                            hoN[j][0:64, ds(qc * QCH, QCH)], a[0:64, :], rp[0:64, :]
                        )
                    else:
                        st0 = npool.tile([128, QCH], BF16, tag="st0", name=f"n{qc}_{h}")
                        nc.vector.tensor_mul(st0[0:64, :], a[0:64, :], rp[0:64, :])
                        nc.vector.tensor_copy(
                            hoN[j][64:128, ds(qc * QCH, QCH)], st0[0:64, :]
                        )

                # ---- output projection for this q chunk ----
                for nch2 in range(QCH // 512):
                    qs = ds(qc * QCH + nch2 * 512, 512)
                    for et in range(NET):
                        p = psp.tile([128, 512], F32, tag="ps")
                        for ct in range(NCT):
                            nc.tensor.matmul(
                                p[:],
                                lhsT=wo_bf[ct][:, ts(et, 128)],
                                rhs=hoN[ct][:, qs],
                                start=(ct == 0),
                                stop=(ct == NCT - 1),
                            )
                        ot = outp.tile([128, 512], F32, tag="out")
                        if et % 2 == 0:
                            nc.vector.tensor_copy(ot[:], p[:])
                        else:
                            nc.scalar.copy(ot[:], p[:])
                        nc.sync.dma_start(yT[ts(et, 128), qs], ot[:])
    return nc


_CACHE = {}


def compiled_nc():
    if "nc" not in _CACHE:
        nc = bacc.Bacc(
            "TRN2", target_bir_lowering=False, debug=False, num_devices=N_CORES
        )
        build_program(nc)
        nc.compile()
        _CACHE["nc"] = nc
    return _CACHE["nc"]


def make_in_maps(query, key, value, Wq, bq, Wk, bk, Wv, bv, Wo, bo):
    f = lambda a: np.ascontiguousarray(np.asarray(a, dtype=np.float32))
    query, key, value = f(query), f(key), f(value)
    Wq, bq, Wk, bk, Wv, Wo = f(Wq), f(bq), f(Wk), f(bk), f(Wv), f(Wo)
    in_maps = []
    for c in range(N_CORES):
        b, g = divmod(c, 2)
        cs = slice(g * C, (g + 1) * C)
        bq_g, bk_g = bq[cs], bk[cs]
        bq4_ = np.ascontiguousarray(bq_g.reshape(NCT, 128).T)
        row = np.array(
            [bq_g[h * D : (h + 1) * D] @ bk_g[h * D : (h + 1) * D] for h in range(HL)],
            np.float32,
        ) * 0.125
        bqbk8_ = np.ascontiguousarray(np.tile(row[None, :], (128, 1)))
        in_maps.append(
            {
                "xq": query[b], "xk": key[b], "xv": value[b],
                "wq": np.ascontiguousarray(Wq[:, cs]),
                "wk": np.ascontiguousarray(Wk[:, cs]),
                "wv": np.ascontiguousarray(Wv[:, cs]),
                "wo": np.ascontiguousarray(Wo[cs, :]),
                "bq4": bq4_, "bqbk8": bqbk8_,
            }
        )
    return in_maps


def assemble(results, bv, bo, Wo):
    extra = (np.asarray(bv, np.float32) @ np.asarray(Wo, np.float32)
             + np.asarray(bo, np.float32))
    out = np.empty((4, S, E), np.float32)
    for b in range(4):
        out[b] = (results[2 * b]["yT"] + results[2 * b + 1]["yT"]).T + extra
    return out


def kernel(query, key, value, Wq, bq, Wk, bk, Wv, bv, Wo, bo):
    nc = compiled_nc()
    in_maps = make_in_maps(query, key, value, Wq, bq, Wk, bk, Wv, bv, Wo, bo)
    res = bass_utils.run_bass_kernel_spmd(nc, in_maps, list(range(N_CORES)))
    return assemble(res.results, bv, bo, Wo)
